# revision 13
# baseline (speedup 1.0000x reference)
"""Trainium2 Bass kernel for nn_MessagePassingNet (gnn_message_passing).

kernel(**inputs) -> [4096, 16] f32 molecule outputs.

Strategy (8 NeuronCores, SPMD, dst-sharded):
- Core c owns atoms [c*16384, (c+1)*16384) and all edges into them.
- Host: sort edges by dst atom, pad each atom's edge list to a multiple
  of 4 slots; pack whole molecules (32 atoms) into 156 blocks of 1024
  slots (LPT + repair).  Emit the per-edge feature stream transposed
  bf16 (rows 0-63 = x[dst], 64-127 = x[src]), a per-block group->atom
  one-hot (4 pieces of [128,64] per block), and per-atom pad counts.
- Device per block (= super-tile of 1024 slots, pair-packed 2x512):
  L1/L2/L3 message MLP on TensorE (block-diag stationaries, biases on
  ACT/DVE/Pool), group-of-4 partial sums via a 2-step strided add tree,
  one PE transpose -> PT [group, h], 4 one-hot scatter matmuls
  accumulate new_states directly in readout layout [2*64 h, atom cols]
  in PSUM.  Pad contamination removed by rank-1 (c_pad x padcnt)
  matmuls that also zero-init each chunk bank.  Readout MLP uses
  block-diagonal weights (2 atoms per column); 32-atom molecule sums
  via tensor_reduce.  Natural atom order end-to-end (no permutation).
"""
import sys
import numpy as np
import ml_dtypes

sys.path.insert(0, "/opt/trn_rl_repo")

from contextlib import ExitStack

import concourse.bass as bass
import concourse.bacc as bacc
import concourse.tile as tile
from concourse import mybir
from concourse.bass_utils import run_bass_kernel_spmd

F32 = mybir.dt.float32
BF16 = mybir.dt.bfloat16
BF = ml_dtypes.bfloat16

N_CORES = 8
D = 64
OUT = 16
ATOMS_PER_MOL = 32


class Cfg:
    """Geometry. Full problem: NB=156 blocks -> 16384 atoms/core."""

    def __init__(self, nb=156, apc=16384):
        self.NB = nb                     # blocks = super-tiles, 1024 slots each
        self.APC = apc                   # atoms per core
        self.MPC = self.APC // ATOMS_PER_MOL
        self.E_CAP = self.NB * 1024      # slot stream length
        self.NCH = (self.NB + 7) // 8    # readout chunks (8 blocks each)
        self.LAST_CB = self.NB - 8 * (self.NCH - 1)  # blocks in last chunk

    @property
    def n_atoms(self):
        return self.APC * N_CORES


FULL = Cfg(156, 16384)


# ---------------------------------------------------------------- host prep

def prep_core(cfg, c, x_bf, edge_src, edge_dst):
    """Build feat [128, E_CAP] bf16, smat [128, NB*256] bf16,
    pads [2, NCH*512] bf16, and the per-atom (block, aib) placement.
    Blocks = greedy consecutive atom ranges (<=1024 slots, <=128 atoms);
    molecule sums happen on the host from the per-atom outputs."""
    lo = c * cfg.APC
    emask = (edge_dst >= lo) & (edge_dst < lo + cfg.APC)
    src_c = edge_src[emask]
    dst_c = edge_dst[emask] - lo

    order = np.argsort(dst_c, kind="stable")
    src_s = src_c[order]
    dst_s = dst_c[order]

    deg = np.bincount(dst_c, minlength=cfg.APC).astype(np.int64)
    padded = 4 * ((deg + 3) // 4)
    pad = padded - deg                                    # 0..3 per atom

    # greedy block cuts over natural atom order
    blk_of_atom = np.zeros(cfg.APC, np.int64)
    aib_of_atom = np.zeros(cfg.APC, np.int64)
    astart = np.zeros(cfg.APC, np.int64)
    b = 0
    used = 0
    natm = 0
    for a in range(cfg.APC):
        s = int(padded[a])
        if used + s > 1024 or natm == 128:
            b += 1
            used = 0
            natm = 0
        blk_of_atom[a] = b
        aib_of_atom[a] = natm
        astart[a] = 1024 * b + used
        used += s
        natm += 1
    assert b < cfg.NB, f"needs {b + 1} blocks > {cfg.NB}"

    # real-edge slot positions
    first_edge = np.cumsum(deg) - deg
    rank = np.arange(len(dst_s)) - first_edge[dst_s]
    slotpos = astart[dst_s] + rank

    feat = np.zeros((128, cfg.E_CAP), BF)
    feat[0:64, slotpos] = x_bf[lo + dst_s].T
    feat[64:128, slotpos] = x_bf[src_s].T

    # group -> atom (pads inherit their atom; block slack = -1)
    atom_of_slot = np.full(cfg.E_CAP, -1, np.int64)
    tot = int(padded.sum())
    cum = np.cumsum(padded) - padded
    r = np.arange(tot) - np.repeat(cum, padded)
    slots_all = np.repeat(astart, padded) + r
    atom_of_slot[slots_all] = np.repeat(np.arange(cfg.APC), padded)
    gatom = atom_of_slot[0::4]
    assert (atom_of_slot[3::4] == gatom).all()

    # one-hot scatter pieces: per block s, cols [256s,256s+256):
    # [0:64]=(A,up) [64:128]=(A,lo) [128:192]=(B,up) [192:256]=(B,lo)
    smat = np.zeros((128, cfg.NB * 256), BF)
    g = np.arange(cfg.NB * 256)
    valid = gatom >= 0
    gv = g[valid]
    av = gatom[valid]
    s_blk = gv // 256
    rel = gv % 256
    half = rel // 128            # 0 = A (slots 0-511), 1 = B
    grow = rel % 128
    aib = aib_of_atom[av]
    upper = (aib < 64).astype(np.int64)
    colbase = 256 * s_blk + 128 * half + 64 * (1 - upper)
    smat[grow, colbase + (aib % 64)] = 1.0

    # pad counts by (chunk psum position): row 0 = upper atoms, row 1 = lower
    pads = np.zeros((2, cfg.NCH * 512), BF)
    ch = blk_of_atom // 8
    j = blk_of_atom % 8
    col = 512 * ch + 64 * j + (aib_of_atom % 64)
    rows = (aib_of_atom >= 64).astype(np.int64)
    pads[rows, col] = pad.astype(BF)
    pads = pads.reshape(1, -1)

    # per-atom position in the [32, NCH*512] device output
    o_row = 16 * rows                     # 0 or 16 (16 OUT dims follow)
    o_col = col
    return feat, smat, pads, (o_row, o_col)


def make_weight_inputs(ws):
    """Shared (replicated) weight tensors in device layouts."""
    def dd(a):
        z = np.zeros((128, a.shape[1] * 2), np.float32)
        z[0:64, 0:a.shape[1]] = a
        z[64:128, a.shape[1]:] = a
        return z

    # device-exact pad message m_pad = MLP(0) in bf16 chain
    b0 = ws["ms0_b"].astype(np.float32)
    h1p = np.maximum(b0, 0).astype(BF)
    w1b = ws["ms1_w"].astype(BF).astype(np.float32)
    h2p = np.maximum(w1b.T @ h1p.astype(np.float32) + ws["ms1_b"], 0).astype(BF)
    w2b = ws["ms2_w"].astype(BF).astype(np.float32)
    mpad = np.maximum(w2b.T @ h2p.astype(np.float32) + ws["ms2_b"], 0).astype(BF)

    return {
        "w0": ws["ms0_w"].astype(BF),                       # [128, 64]
        "wdiag1": dd(ws["ms1_w"]).astype(BF),               # [128, 128]
        "w2ext": dd(ws["ms2_w"]).astype(BF),                # [128, 128]
        "fc1dd": dd(ws["fc1_w"]).astype(BF),                # [128, 128]
        "fc2dd": dd(ws["fc2_w"]).astype(BF),                # [128, 128]
        "owdd": dd(ws["out_w"]).astype(BF),                 # [128, 32]
        "b0d": np.concatenate([ws["ms0_b"], ws["ms0_b"]])[:, None].astype(np.float32),
        "b1d": np.concatenate([ws["ms1_b"], ws["ms1_b"]])[:, None].astype(np.float32),
        "b2d": np.concatenate([ws["ms2_b"], ws["ms2_b"]])[:, None].astype(np.float32),
        "fb1d": np.concatenate([ws["fc1_b"], ws["fc1_b"]])[:, None].astype(np.float32),
        "fb2d": np.concatenate([ws["fc2_b"], ws["fc2_b"]])[:, None].astype(np.float32),
        "obd": np.concatenate([ws["out_b"], ws["out_b"]])[:, None].astype(np.float32),
        "cpadneg": (-mpad.astype(np.float32)).astype(BF)[None, :],  # [1, 64]
        "ident": np.eye(128, dtype=np.float32).astype(BF),
    }


# ------------------------------------------------------------- device build

def build(cfg):
    nc = bacc.Bacc(None, target_bir_lowering=False)
    Relu = mybir.ActivationFunctionType.Relu
    Copy = mybir.ActivationFunctionType.Copy
    Add = mybir.AluOpType.add
    Max = mybir.AluOpType.max

    feat_d = nc.declare_dram_parameter("feat", [128, cfg.E_CAP], BF16, isOutput=False)
    smat_d = nc.declare_dram_parameter("smat", [128, cfg.NB * 256], BF16, isOutput=False)
    pads_d = nc.declare_dram_parameter("pads", [1, 2 * cfg.NCH * 512], BF16, isOutput=False)
    w0_d = nc.declare_dram_parameter("w0", [128, 64], BF16, isOutput=False)
    wdiag1_d = nc.declare_dram_parameter("wdiag1", [128, 128], BF16, isOutput=False)
    w2ext_d = nc.declare_dram_parameter("w2ext", [128, 128], BF16, isOutput=False)
    fc1dd_d = nc.declare_dram_parameter("fc1dd", [128, 128], BF16, isOutput=False)
    fc2dd_d = nc.declare_dram_parameter("fc2dd", [128, 128], BF16, isOutput=False)
    owdd_d = nc.declare_dram_parameter("owdd", [128, 32], BF16, isOutput=False)
    b0d_d = nc.declare_dram_parameter("b0d", [128, 1], F32, isOutput=False)
    b1d_d = nc.declare_dram_parameter("b1d", [128, 1], F32, isOutput=False)
    b2d_d = nc.declare_dram_parameter("b2d", [128, 1], F32, isOutput=False)
    fb1d_d = nc.declare_dram_parameter("fb1d", [128, 1], F32, isOutput=False)
    fb2d_d = nc.declare_dram_parameter("fb2d", [128, 1], F32, isOutput=False)
    obd_d = nc.declare_dram_parameter("obd", [32, 1], F32, isOutput=False)
    cpadneg_d = nc.declare_dram_parameter("cpadneg", [1, 64], BF16, isOutput=False)
    ident_d = nc.declare_dram_parameter("ident", [128, 128], BF16, isOutput=False)
    oat_d = nc.declare_dram_parameter("oat", [32, cfg.NCH * 512], F32, isOutput=True)

    FCH = 4                      # STs per feat/smat DMA chunk
    NFC = (cfg.NB + FCH - 1) // FCH

    with tile.TileContext(nc) as tc, ExitStack() as octx:
        const = octx.enter_context(tc.tile_pool(name="const", bufs=1))

        def cload(name, dram, shape, dt):
            t = const.tile(shape, dt, name=name)
            nc.sync.dma_start(out=t[:], in_=dram[:])
            return t

        w0 = cload("w0", w0_d, [128, 64], BF16)
        wdiag1 = cload("wdiag1", wdiag1_d, [128, 128], BF16)
        w2ext = cload("w2ext", w2ext_d, [128, 128], BF16)
        fc1dd = cload("fc1dd", fc1dd_d, [128, 128], BF16)
        fc2dd = cload("fc2dd", fc2dd_d, [128, 128], BF16)
        owdd = cload("owdd", owdd_d, [128, 32], BF16)
        b0d = cload("b0d", b0d_d, [128, 1], F32)
        b1d = cload("b1d", b1d_d, [128, 1], F32)
        b2d = cload("b2d", b2d_d, [128, 1], F32)
        fb1d = cload("fb1d", fb1d_d, [128, 1], F32)
        fb2d = cload("fb2d", fb2d_d, [128, 1], F32)
        obd = cload("obd", obd_d, [32, 1], F32)
        cpadneg = cload("cpadneg", cpadneg_d, [1, 64], BF16)
        ident = cload("ident", ident_d, [128, 128], BF16)
        pads = cload("pads", pads_d, [1, 2 * cfg.NCH * 512], BF16)

        with ExitStack() as ctx:
            featp = ctx.enter_context(tc.tile_pool(name="featp", bufs=3))
            scp = ctx.enter_context(tc.tile_pool(name="scp", bufs=3))
            hp = ctx.enter_context(tc.tile_pool(name="hp", bufs=2))
            pph1 = ctx.enter_context(tc.tile_pool(name="pph1", bufs=2, space="PSUM"))
            pwork = ctx.enter_context(tc.tile_pool(name="pwork", bufs=3, space="PSUM"))
            pchunk = ctx.enter_context(tc.tile_pool(name="pchunk", bufs=2, space="PSUM"))
            prd = ctx.enter_context(tc.tile_pool(name="prd", bufs=1, space="PSUM"))
            rp = ctx.enter_context(tc.tile_pool(name="rp", bufs=2))

            NPAD = cfg.NCH * 512
            fc_tiles = {}
            sc_tiles = {}

            def load_chunks(k):
                if k < NFC and k not in fc_tiles:
                    lo_c = k * FCH * 1024
                    n = min(FCH * 1024, cfg.E_CAP - lo_c)
                    ft = featp.tile([128, FCH * 1024], BF16, tag="featc", name="featc")
                    eng = nc.sync if k % 2 == 0 else nc.scalar
                    eng.dma_start(out=ft[:, :n], in_=feat_d[:, lo_c:lo_c + n])
                    fc_tiles[k] = ft
                    lo_s = k * FCH * 256
                    ns = min(FCH * 256, cfg.NB * 256 - lo_s)
                    st = scp.tile([128, FCH * 256], BF16, tag="scc", name="scc")
                    nc.sync.dma_start(out=st[:, :ns], in_=smat_d[:, lo_s:lo_s + ns])
                    sc_tiles[k] = st

            load_chunks(0)
            load_chunks(1)
            load_chunks(2)

            # per-stage state handles
            ph1s, h1s, ph2s, h2s, pms, ms = {}, {}, {}, {}, {}, {}
            t1s, p4s, pts, ptss = {}, {}, {}, {}
            chunkps, nsts, hr1s, hr2s, o_s = {}, {}, {}, {}, {}

            def cb(c):
                """blocks in chunk c"""
                return 8 if c < cfg.NCH - 1 else cfg.LAST_CB

            def stage_A(n):  # L1 pair + feat prefetch
                if n % FCH == 0:
                    load_chunks(n // FCH + 3)
                fcol = (n % FCH) * 1024
                featc = fc_tiles[n // FCH]
                ph1 = pph1.tile([128, 512], F32, tag="ph1", name="ph1")
                nc.tensor.matmul(out=ph1[0:64, :], lhsT=w0[:],
                                 rhs=featc[:, fcol:fcol + 512],
                                 start=True, stop=True)
                nc.tensor.matmul(out=ph1[64:128, :], lhsT=w0[:],
                                 rhs=featc[:, fcol + 512:fcol + 1024],
                                 start=True, stop=True)
                ph1s[n] = ph1
                h1 = hp.tile([128, 512], BF16, tag="h1", name="h1")
                nc.scalar.activation(out=h1[:], in_=ph1[:], func=Relu,
                                     bias=b0d[:])
                h1s[n] = h1
                del ph1s[n]

            def stage_B(n):  # L2
                ph2 = pwork.tile([128, 512], F32, tag="wk", name="ph2")
                nc.tensor.matmul(out=ph2[:], lhsT=wdiag1[:], rhs=h1s.pop(n)[:],
                                 start=True, stop=True)
                h2 = hp.tile([128, 512], BF16, tag="h2", name="h2")
                nc.vector.tensor_scalar(out=h2[:], in0=ph2[:], scalar1=b1d[:],
                                        scalar2=0.0, op0=Add, op1=Max)
                h2s[n] = h2

            def stage_C(n):  # L3 + m relu + add tree
                pm = pwork.tile([128, 512], F32, tag="wk", name="pm")
                nc.tensor.matmul(out=pm[:], lhsT=w2ext[:], rhs=h2s.pop(n)[:],
                                 start=True, stop=True)
                m = hp.tile([128, 512], BF16, tag="m", name="m")
                nc.scalar.activation(out=m[:, 0:320], in_=pm[:, 0:320],
                                     func=Relu, bias=b2d[:])
                nc.vector.tensor_scalar(out=m[:, 320:512], in0=pm[:, 320:512],
                                        scalar1=b2d[:], scalar2=0.0,
                                        op0=Add, op1=Max)
                t1 = hp.tile([128, 256], BF16, tag="t1", name="t1")
                m3 = m[:].rearrange("p (g k) -> p g k", k=4)
                t13 = t1[:].rearrange("p (g k) -> p g k", k=2)
                nc.gpsimd.tensor_tensor(out=t13, in0=m3[:, :, 0:2],
                                        in1=m3[:, :, 2:4], op=Add)
                p4 = hp.tile([128, 128], BF16, tag="p4", name="p4")
                p43 = p4[:].rearrange("p (g k) -> p g k", k=1)
                nc.gpsimd.tensor_tensor(out=p43, in0=t13[:, :, 0:1],
                                        in1=t13[:, :, 1:2], op=Add)
                p4s[n] = p4

            def stage_D(n):  # transpose + copy to sbuf
                pt = pwork.tile([128, 128], BF16, tag="wk", name="pt")
                nc.tensor.transpose(out=pt[:], in_=p4s.pop(n)[:], identity=ident[:])
                pts_t = hp.tile([128, 128], BF16, tag="pts", name="pts")
                nc.vector.tensor_copy(out=pts_t[:], in_=pt[:])
                ptss[n] = pts_t

            def stage_corr(c):  # zero-init + pad correction for chunk c
                w = 64 * cb(c)
                cp = pchunk.tile([128, 512], F32, tag="ck", name="chunkp")
                nc.tensor.matmul(out=cp[0:64, :w], lhsT=cpadneg[:],
                                 rhs=pads[0:1, 512 * c:512 * c + w],
                                 start=True, stop=False, skip_group_check=True)
                nc.tensor.matmul(out=cp[64:128, :w], lhsT=cpadneg[:],
                                 rhs=pads[0:1, NPAD + 512 * c:NPAD + 512 * c + w],
                                 start=True, stop=False, skip_group_check=True)
                chunkps[c] = cp

            def stage_E(n):  # scatter into chunk psum
                c, j = n // 8, n % 8
                cp = chunkps[c]
                scc = sc_tiles[n // FCH]
                so = (n % FCH) * 256
                pts_t = ptss.pop(n)
                last = n == min(8 * c + 7, cfg.NB - 1)
                for piece in range(4):
                    half, lowr = piece // 2, piece % 2
                    nc.tensor.matmul(
                        out=cp[64 * lowr:64 * lowr + 64, 64 * j:64 * j + 64],
                        lhsT=pts_t[:, 64 * half:64 * half + 64],
                        rhs=scc[:, so + 64 * piece:so + 64 * piece + 64],
                        start=False, stop=bool(last and piece >= 2),
                        skip_group_check=True)
                if n // FCH in sc_tiles and (n + 1) % FCH == 0:
                    del sc_tiles[n // FCH]
                    if n // FCH - 1 in fc_tiles:
                        del fc_tiles[n // FCH - 1]

            def stage_nst(c):  # chunk psum -> sbuf
                w = 64 * cb(c)
                nst = rp.tile([128, 512], BF16, tag="nst", name="nst")
                nc.scalar.activation(out=nst[:, :w], in_=chunkps.pop(c)[:, :w],
                                     func=Copy)
                nsts[c] = nst

            def stage_fc1(c):
                w = 64 * cb(c)
                pr = prd.tile([128, 512], F32, tag="rd", name="prfc1")
                nc.tensor.matmul(out=pr[:, :w], lhsT=fc1dd[:],
                                 rhs=nsts.pop(c)[:, :w], start=True, stop=True)
                hr1 = rp.tile([128, 512], BF16, tag="hr1", name="hr1")
                nc.scalar.activation(out=hr1[:, :w], in_=pr[:, :w], func=Relu,
                                     bias=fb1d[:])
                hr1s[c] = hr1

            def stage_fc2(c):
                w = 64 * cb(c)
                pr = prd.tile([128, 512], F32, tag="rd", name="prfc2")
                nc.tensor.matmul(out=pr[:, :w], lhsT=fc2dd[:],
                                 rhs=hr1s.pop(c)[:, :w], start=True, stop=True)
                hr2 = rp.tile([128, 512], BF16, tag="hr2", name="hr2")
                nc.scalar.activation(out=hr2[:, :w], in_=pr[:, :w], func=Relu,
                                     bias=fb2d[:])
                hr2s[c] = hr2

            def stage_out(c):
                w = 64 * cb(c)
                pr = prd.tile([128, 512], F32, tag="rd", name="prout")
                nc.tensor.matmul(out=pr[0:32, :w], lhsT=owdd[:],
                                 rhs=hr2s.pop(c)[:, :w], start=True, stop=True)
                o = rp.tile([32, 512], F32, tag="o", name="o")
                nc.vector.tensor_scalar(out=o[:, :w], in0=pr[0:32, :w],
                                        scalar1=obd[0:32, :], scalar2=0.0,
                                        op0=Add, op1=Max)
                nc.sync.dma_start(out=oat_d[:, 512 * c:512 * c + w],
                                  in_=o[:, :w])

            # software-pipelined main loop
            for i in range(cfg.NB + 8):
                if i % 8 == 4 and i // 8 < cfg.NCH:
                    stage_corr(i // 8)        # before first scatter of chunk
                if 0 <= i < cfg.NB:
                    stage_A(i)
                if 0 <= i - 1 < cfg.NB:
                    stage_B(i - 1)
                if 0 <= i - 2 < cfg.NB:
                    stage_C(i - 2)
                if 0 <= i - 4 < cfg.NB:
                    stage_E(i - 4)
                if 0 <= i - 3 < cfg.NB:
                    stage_D(i - 3)
                n_done = i - 4                # highest block whose scatter done
                for c in range(cfg.NCH):
                    lastb = min(8 * c + 7, cfg.NB - 1)
                    if n_done == lastb:
                        stage_nst(c)
                    elif n_done == lastb + 1:
                        stage_fc1(c)
                    elif n_done == lastb + 2:
                        stage_fc2(c)
                    elif n_done == lastb + 3:
                        stage_out(c)

    nc.compile()
    return nc


# ------------------------------------------------------------------ runner

_CACHE = {}


def _get_nc(cfg):
    key = (cfg.NB, cfg.APC)
    if key not in _CACHE:
        _CACHE[key] = build(cfg)
    return _CACHE[key]


def run(cfg, inputs, trace=False, tmpdir=None):
    ws = {k: np.asarray(v) for k, v in inputs.items()}
    x_bf = ws["atom_states"].astype(BF)
    shared = make_weight_inputs(ws)

    in_maps = []
    omaps = []
    for c in range(N_CORES):
        feat, smat, pads, omap = prep_core(
            cfg, c, x_bf, ws["edge_src"], ws["edge_dst"])
        m = dict(shared)
        m["feat"] = feat
        m["smat"] = smat
        m["pads"] = pads
        in_maps.append(m)
        omaps.append(omap)

    nc = _get_nc(cfg)
    kw = {}
    if trace:
        kw = dict(trace=True, tmpdir=tmpdir)
    r = run_bass_kernel_spmd(nc, in_maps, list(range(N_CORES)), **kw)

    out = np.zeros((cfg.MPC * N_CORES, OUT), np.float32)
    for c in range(N_CORES):
        oat = r.results[c]["oat"]                  # [32, NCH*512]
        o_row, o_col = omaps[c]
        per_atom = oat[o_row[:, None] + np.arange(OUT)[None, :], o_col[:, None]]
        out[c * cfg.MPC:(c + 1) * cfg.MPC] = \
            per_atom.reshape(cfg.MPC, ATOMS_PER_MOL, OUT).sum(1)
    return out, r


def kernel(**inputs) -> np.ndarray:
    out, _ = run(FULL, inputs)
    return out


# revision 15
# speedup vs baseline: 1.4481x; 1.4481x over previous
"""Trainium2 Bass kernel for nn_MessagePassingNet (gnn_message_passing).

kernel(**inputs) -> [4096, 16] f32 molecule outputs.

Strategy (8 NeuronCores, SPMD, dst-sharded):
- Core c owns atoms [c*16384, (c+1)*16384) and all edges into them.
- Host: sort edges by dst atom, pad each atom's edge list to a multiple
  of 4 slots; pack whole molecules (32 atoms) into 156 blocks of 1024
  slots (LPT + repair).  Emit the per-edge feature stream transposed
  bf16 (rows 0-63 = x[dst], 64-127 = x[src]), a per-block group->atom
  one-hot (4 pieces of [128,64] per block), and per-atom pad counts.
- Device per block (= super-tile of 1024 slots, pair-packed 2x512):
  L1/L2/L3 message MLP on TensorE (block-diag stationaries, biases on
  ACT/DVE/Pool), group-of-4 partial sums via a 2-step strided add tree,
  one PE transpose -> PT [group, h], 4 one-hot scatter matmuls
  accumulate new_states directly in readout layout [2*64 h, atom cols]
  in PSUM.  Pad contamination removed by rank-1 (c_pad x padcnt)
  matmuls that also zero-init each chunk bank.  Readout MLP uses
  block-diagonal weights (2 atoms per column); 32-atom molecule sums
  via tensor_reduce.  Natural atom order end-to-end (no permutation).
"""
import sys
import numpy as np
import ml_dtypes

sys.path.insert(0, "/opt/trn_rl_repo")

from contextlib import ExitStack

import concourse.bass as bass
import concourse.bacc as bacc
import concourse.tile as tile
from concourse import mybir
from concourse.bass_utils import run_bass_kernel_spmd

F32 = mybir.dt.float32
BF16 = mybir.dt.bfloat16
BF = ml_dtypes.bfloat16

N_CORES = 8
D = 64
OUT = 16
ATOMS_PER_MOL = 32


class Cfg:
    """Geometry. Full problem: NB=156 blocks -> 16384 atoms/core."""

    def __init__(self, nb=156, apc=16384):
        self.NB = nb                     # blocks = super-tiles, 1024 slots each
        self.APC = apc                   # atoms per core
        self.MPC = self.APC // ATOMS_PER_MOL
        self.E_CAP = self.NB * 1024      # slot stream length
        self.NCH = (self.NB + 7) // 8    # readout chunks (8 blocks each)
        self.LAST_CB = self.NB - 8 * (self.NCH - 1)  # blocks in last chunk

    @property
    def n_atoms(self):
        return self.APC * N_CORES


FULL = Cfg(156, 16384)


# ---------------------------------------------------------------- host prep

def prep_core(cfg, c, x_bf, edge_src, edge_dst):
    """Build feat [128, E_CAP] bf16, smat [128, NB*256] bf16,
    pads [2, NCH*512] bf16, and the per-atom (block, aib) placement.
    Blocks = greedy consecutive atom ranges (<=1024 slots, <=128 atoms);
    molecule sums happen on the host from the per-atom outputs."""
    lo = c * cfg.APC
    emask = (edge_dst >= lo) & (edge_dst < lo + cfg.APC)
    src_c = edge_src[emask]
    dst_c = edge_dst[emask] - lo

    order = np.argsort(dst_c, kind="stable")
    src_s = src_c[order]
    dst_s = dst_c[order]

    deg = np.bincount(dst_c, minlength=cfg.APC).astype(np.int64)
    padded = 4 * ((deg + 3) // 4)
    pad = padded - deg                                    # 0..3 per atom

    # greedy half-block cuts (512 slots, <=64 atoms) over natural atom order
    hb_of_atom = np.zeros(cfg.APC, np.int64)
    aib_of_atom = np.zeros(cfg.APC, np.int64)
    astart = np.zeros(cfg.APC, np.int64)
    h = 0
    used = 0
    natm = 0
    for a in range(cfg.APC):
        s = int(padded[a])
        if used + s > 512 or natm == 64:
            h += 1
            used = 0
            natm = 0
        hb_of_atom[a] = h
        aib_of_atom[a] = natm
        astart[a] = 512 * h + used
        used += s
        natm += 1
    assert h < 2 * cfg.NB, f"needs {h + 1} half-blocks > {2 * cfg.NB}"

    # real-edge slot positions
    first_edge = np.cumsum(deg) - deg
    rank = np.arange(len(dst_s)) - first_edge[dst_s]
    slotpos = astart[dst_s] + rank

    feat = np.zeros((128, cfg.E_CAP), BF)
    feat[0:64, slotpos] = x_bf[lo + dst_s].T
    feat[64:128, slotpos] = x_bf[src_s].T

    # group -> atom (pads inherit their atom; block slack = -1)
    atom_of_slot = np.full(cfg.E_CAP, -1, np.int64)
    tot = int(padded.sum())
    cum = np.cumsum(padded) - padded
    r = np.arange(tot) - np.repeat(cum, padded)
    slots_all = np.repeat(astart, padded) + r
    atom_of_slot[slots_all] = np.repeat(np.arange(cfg.APC), padded)
    gatom = atom_of_slot[0::4]
    assert (atom_of_slot[3::4] == gatom).all()

    # one-hot scatter pieces: per ST s, cols [128s,128s+128):
    # [0:64] = half-block 2s (psum upper), [64:128] = half-block 2s+1 (lower)
    smat = np.zeros((128, cfg.NB * 128), BF)
    g = np.arange(cfg.NB * 256)
    valid = gatom >= 0
    gv = g[valid]
    av = gatom[valid]
    hbv = gv // 128              # half-block of group
    grow = gv % 128              # group row within half
    aib = aib_of_atom[av]
    assert (hbv == hb_of_atom[av]).all()
    colbase = 128 * (hbv // 2) + 64 * (hbv % 2)
    smat[grow, colbase + aib] = 1.0

    # pad counts by (chunk psum position): row 0 = upper (even hb), 1 = lower
    pads = np.zeros((2, cfg.NCH * 512), BF)
    st = hb_of_atom // 2
    ch = st // 8
    j = st % 8
    col = 512 * ch + 64 * j + aib_of_atom
    rows = (hb_of_atom % 2).astype(np.int64)
    pads[rows, col] = pad.astype(BF)
    pads = pads.reshape(1, -1)

    # per-atom position in the [32, NCH*512] device output
    o_row = 16 * rows                     # 0 or 16 (16 OUT dims follow)
    o_col = col
    return feat, smat, pads, (o_row, o_col)


def make_weight_inputs(ws):
    """Shared (replicated) weight tensors in device layouts."""
    def dd(a):
        z = np.zeros((128, a.shape[1] * 2), np.float32)
        z[0:64, 0:a.shape[1]] = a
        z[64:128, a.shape[1]:] = a
        return z

    # device-exact pad message m_pad = MLP(0) in bf16 chain
    b0 = ws["ms0_b"].astype(np.float32)
    h1p = np.maximum(b0, 0).astype(BF)
    w1b = ws["ms1_w"].astype(BF).astype(np.float32)
    h2p = np.maximum(w1b.T @ h1p.astype(np.float32) + ws["ms1_b"], 0).astype(BF)
    w2b = ws["ms2_w"].astype(BF).astype(np.float32)
    mpad = np.maximum(w2b.T @ h2p.astype(np.float32) + ws["ms2_b"], 0).astype(BF)

    return {
        "w0": ws["ms0_w"].astype(BF),                       # [128, 64]
        "wdiag1": dd(ws["ms1_w"]).astype(BF),               # [128, 128]
        "w2ext": dd(ws["ms2_w"]).astype(BF),                # [128, 128]
        "fc1dd": dd(ws["fc1_w"]).astype(BF),                # [128, 128]
        "fc2dd": dd(ws["fc2_w"]).astype(BF),                # [128, 128]
        "owdd": dd(ws["out_w"]).astype(BF),                 # [128, 32]
        "b0d": np.concatenate([ws["ms0_b"], ws["ms0_b"]])[:, None].astype(np.float32),
        "b1d": np.concatenate([ws["ms1_b"], ws["ms1_b"]])[:, None].astype(np.float32),
        "b2d": np.concatenate([ws["ms2_b"], ws["ms2_b"]])[:, None].astype(np.float32),
        "fb1d": np.concatenate([ws["fc1_b"], ws["fc1_b"]])[:, None].astype(np.float32),
        "fb2d": np.concatenate([ws["fc2_b"], ws["fc2_b"]])[:, None].astype(np.float32),
        "obd": np.concatenate([ws["out_b"], ws["out_b"]])[:, None].astype(np.float32),
        "cpadneg": (-mpad.astype(np.float32)).astype(BF)[None, :],  # [1, 64]
        "ident": np.eye(128, dtype=np.float32).astype(BF),
    }


# ------------------------------------------------------------- device build

def build(cfg):
    nc = bacc.Bacc(None, target_bir_lowering=False)
    Relu = mybir.ActivationFunctionType.Relu
    Copy = mybir.ActivationFunctionType.Copy
    Add = mybir.AluOpType.add
    Max = mybir.AluOpType.max

    feat_d = nc.declare_dram_parameter("feat", [128, cfg.E_CAP], BF16, isOutput=False)
    smat_d = nc.declare_dram_parameter("smat", [128, cfg.NB * 128], BF16, isOutput=False)
    pads_d = nc.declare_dram_parameter("pads", [1, 2 * cfg.NCH * 512], BF16, isOutput=False)
    w0_d = nc.declare_dram_parameter("w0", [128, 64], BF16, isOutput=False)
    wdiag1_d = nc.declare_dram_parameter("wdiag1", [128, 128], BF16, isOutput=False)
    w2ext_d = nc.declare_dram_parameter("w2ext", [128, 128], BF16, isOutput=False)
    fc1dd_d = nc.declare_dram_parameter("fc1dd", [128, 128], BF16, isOutput=False)
    fc2dd_d = nc.declare_dram_parameter("fc2dd", [128, 128], BF16, isOutput=False)
    owdd_d = nc.declare_dram_parameter("owdd", [128, 32], BF16, isOutput=False)
    b0d_d = nc.declare_dram_parameter("b0d", [128, 1], F32, isOutput=False)
    b1d_d = nc.declare_dram_parameter("b1d", [128, 1], F32, isOutput=False)
    b2d_d = nc.declare_dram_parameter("b2d", [128, 1], F32, isOutput=False)
    fb1d_d = nc.declare_dram_parameter("fb1d", [128, 1], F32, isOutput=False)
    fb2d_d = nc.declare_dram_parameter("fb2d", [128, 1], F32, isOutput=False)
    obd_d = nc.declare_dram_parameter("obd", [32, 1], F32, isOutput=False)
    cpadneg_d = nc.declare_dram_parameter("cpadneg", [1, 64], BF16, isOutput=False)
    ident_d = nc.declare_dram_parameter("ident", [128, 128], BF16, isOutput=False)
    oat_d = nc.declare_dram_parameter("oat", [32, cfg.NCH * 512], F32, isOutput=True)

    FCH = 4                      # STs per feat/smat DMA chunk
    NFC = (cfg.NB + FCH - 1) // FCH

    with tile.TileContext(nc) as tc, ExitStack() as octx:
        const = octx.enter_context(tc.tile_pool(name="const", bufs=1))

        def cload(name, dram, shape, dt):
            t = const.tile(shape, dt, name=name)
            nc.sync.dma_start(out=t[:], in_=dram[:])
            return t

        w0 = cload("w0", w0_d, [128, 64], BF16)
        wdiag1 = cload("wdiag1", wdiag1_d, [128, 128], BF16)
        w2ext = cload("w2ext", w2ext_d, [128, 128], BF16)
        fc1dd = cload("fc1dd", fc1dd_d, [128, 128], BF16)
        fc2dd = cload("fc2dd", fc2dd_d, [128, 128], BF16)
        owdd = cload("owdd", owdd_d, [128, 32], BF16)
        b0d = cload("b0d", b0d_d, [128, 1], F32)
        b1d = cload("b1d", b1d_d, [128, 1], F32)
        b2d = cload("b2d", b2d_d, [128, 1], F32)
        fb1d = cload("fb1d", fb1d_d, [128, 1], F32)
        fb2d = cload("fb2d", fb2d_d, [128, 1], F32)
        obd = cload("obd", obd_d, [32, 1], F32)
        cpadneg = cload("cpadneg", cpadneg_d, [1, 64], BF16)
        ident = cload("ident", ident_d, [128, 128], BF16)
        pads = cload("pads", pads_d, [1, 2 * cfg.NCH * 512], BF16)

        with ExitStack() as ctx:
            featp = ctx.enter_context(tc.tile_pool(name="featp", bufs=4))
            scp = ctx.enter_context(tc.tile_pool(name="scp", bufs=4))
            hp = ctx.enter_context(tc.tile_pool(name="hp", bufs=4))
            pph1 = ctx.enter_context(tc.tile_pool(name="pph1", bufs=2, space="PSUM"))
            pwork = ctx.enter_context(tc.tile_pool(name="pwork", bufs=3, space="PSUM"))
            pchunk = ctx.enter_context(tc.tile_pool(name="pchunk", bufs=2, space="PSUM"))
            prd = ctx.enter_context(tc.tile_pool(name="prd", bufs=1, space="PSUM"))
            rp = ctx.enter_context(tc.tile_pool(name="rp", bufs=2))

            NPAD = cfg.NCH * 512
            fc_tiles = {}
            sc_tiles = {}

            def load_feat(k):
                if k < NFC and k not in fc_tiles:
                    lo_c = k * FCH * 1024
                    n = min(FCH * 1024, cfg.E_CAP - lo_c)
                    ft = featp.tile([128, FCH * 1024], BF16, tag="featc", name="featc")
                    eng = nc.sync if k % 2 == 0 else nc.scalar
                    eng.dma_start(out=ft[:, :n], in_=feat_d[:, lo_c:lo_c + n])
                    fc_tiles[k] = ft

            def load_sc(k):
                if k < NFC and k not in sc_tiles:
                    lo_s = k * FCH * 128
                    ns = min(FCH * 128, cfg.NB * 128 - lo_s)
                    st = scp.tile([128, FCH * 128], BF16, tag="scc", name="scc")
                    nc.sync.dma_start(out=st[:, :ns], in_=smat_d[:, lo_s:lo_s + ns])
                    sc_tiles[k] = st

            for _k in range(3):
                load_feat(_k)
                load_sc(_k)

            # per-stage state handles
            ph1s, h1s, ph2s, h2s, pms, ms = {}, {}, {}, {}, {}, {}
            t1s, p4s, pts, ptss = {}, {}, {}, {}
            chunkps, nsts, hr1s, hr2s, o_s = {}, {}, {}, {}, {}

            def cb(c):
                """blocks in chunk c"""
                return 8 if c < cfg.NCH - 1 else cfg.LAST_CB

            def stage_A(n):  # L1 pair + feat prefetch
                if n % FCH == 0:
                    load_feat(n // FCH + 3)
                fcol = (n % FCH) * 1024
                featc = fc_tiles[n // FCH]
                ph1 = pph1.tile([128, 512], F32, tag="ph1", name="ph1")
                nc.tensor.matmul(out=ph1[0:64, :], lhsT=w0[:],
                                 rhs=featc[:, fcol:fcol + 512],
                                 start=True, stop=True)
                nc.tensor.matmul(out=ph1[64:128, :], lhsT=w0[:],
                                 rhs=featc[:, fcol + 512:fcol + 1024],
                                 start=True, stop=True)
                ph1s[n] = ph1
                h1 = hp.tile([128, 512], BF16, tag="h1", name="h1")
                nc.scalar.activation(out=h1[:], in_=ph1[:], func=Relu,
                                     bias=b0d[:])
                h1s[n] = h1
                del ph1s[n]

            def stage_B(n):  # L2
                ph2 = pwork.tile([128, 512], F32, tag="wk", name="ph2")
                nc.tensor.matmul(out=ph2[:], lhsT=wdiag1[:], rhs=h1s.pop(n)[:],
                                 start=True, stop=True)
                h2 = hp.tile([128, 512], BF16, tag="h2", name="h2")
                nc.vector.tensor_scalar(out=h2[:], in0=ph2[:], scalar1=b1d[:],
                                        scalar2=0.0, op0=Add, op1=Max)
                h2s[n] = h2

            def stage_C(n):  # L3 + m relu + add tree
                pm = pwork.tile([128, 512], F32, tag="wk", name="pm")
                nc.tensor.matmul(out=pm[:], lhsT=w2ext[:], rhs=h2s.pop(n)[:],
                                 start=True, stop=True)
                m = hp.tile([128, 512], BF16, tag="m", name="m")
                nc.scalar.activation(out=m[:, 0:272], in_=pm[:, 0:272],
                                     func=Relu, bias=b2d[:])
                nc.vector.tensor_scalar(out=m[:, 272:512], in0=pm[:, 272:512],
                                        scalar1=b2d[:], scalar2=0.0,
                                        op0=Add, op1=Max)
                t1 = hp.tile([128, 256], BF16, tag="t1", name="t1")
                m3 = m[:].rearrange("p (g k) -> p g k", k=4)
                t13 = t1[:].rearrange("p (g k) -> p g k", k=2)
                nc.gpsimd.tensor_tensor(out=t13, in0=m3[:, :, 0:2],
                                        in1=m3[:, :, 2:4], op=Add)
                p4 = hp.tile([128, 128], BF16, tag="p4", name="p4")
                p43 = p4[:].rearrange("p (g k) -> p g k", k=1)
                nc.gpsimd.tensor_tensor(out=p43, in0=t13[:, :, 0:1],
                                        in1=t13[:, :, 1:2], op=Add)
                p4s[n] = p4

            def stage_D(n):  # transpose + copy to sbuf
                pt = pwork.tile([128, 128], BF16, tag="wk", name="pt")
                nc.tensor.transpose(out=pt[:], in_=p4s.pop(n)[:], identity=ident[:])
                pts_t = hp.tile([128, 128], BF16, tag="pts", name="pts")
                nc.vector.tensor_copy(out=pts_t[:], in_=pt[:])
                ptss[n] = pts_t

            def stage_corr(c):  # zero-init + pad correction for chunk c
                w = 64 * cb(c)
                cp = pchunk.tile([128, 512], F32, tag="ck", name="chunkp")
                nc.tensor.matmul(out=cp[0:64, :w], lhsT=cpadneg[:],
                                 rhs=pads[0:1, 512 * c:512 * c + w],
                                 start=True, stop=False, skip_group_check=True)
                nc.tensor.matmul(out=cp[64:128, :w], lhsT=cpadneg[:],
                                 rhs=pads[0:1, NPAD + 512 * c:NPAD + 512 * c + w],
                                 start=True, stop=False, skip_group_check=True)
                chunkps[c] = cp

            def stage_E(n):  # scatter into chunk psum (2 matmuls)
                c, j = n // 8, n % 8
                if n % FCH == 0:
                    load_sc(n // FCH + 3)
                cp = chunkps[c]
                scc = sc_tiles[n // FCH]
                so = (n % FCH) * 128
                pts_t = ptss.pop(n)
                last = n == min(8 * c + 7, cfg.NB - 1)
                nc.tensor.matmul(
                    out=cp[0:64, 64 * j:64 * j + 64],
                    lhsT=pts_t[:, 0:64],
                    rhs=scc[:, so:so + 64],
                    start=False, stop=bool(last), skip_group_check=True)
                nc.tensor.matmul(
                    out=cp[64:128, 64 * j:64 * j + 64],
                    lhsT=pts_t[:, 64:128],
                    rhs=scc[:, so + 64:so + 128],
                    start=False, stop=bool(last), skip_group_check=True)

            def stage_nst(c):  # chunk psum -> sbuf
                w = 64 * cb(c)
                nst = rp.tile([128, 512], BF16, tag="nst", name="nst")
                nc.scalar.activation(out=nst[:, :w], in_=chunkps.pop(c)[:, :w],
                                     func=Copy)
                nsts[c] = nst

            def stage_fc1(c):
                w = 64 * cb(c)
                pr = prd.tile([128, 512], F32, tag="rd", name="prfc1")
                nc.tensor.matmul(out=pr[:, :w], lhsT=fc1dd[:],
                                 rhs=nsts.pop(c)[:, :w], start=True, stop=True)
                hr1 = rp.tile([128, 512], BF16, tag="hr1", name="hr1")
                nc.scalar.activation(out=hr1[:, :w], in_=pr[:, :w], func=Relu,
                                     bias=fb1d[:])
                hr1s[c] = hr1

            def stage_fc2(c):
                w = 64 * cb(c)
                pr = prd.tile([128, 512], F32, tag="rd", name="prfc2")
                nc.tensor.matmul(out=pr[:, :w], lhsT=fc2dd[:],
                                 rhs=hr1s.pop(c)[:, :w], start=True, stop=True)
                hr2 = rp.tile([128, 512], BF16, tag="hr2", name="hr2")
                nc.scalar.activation(out=hr2[:, :w], in_=pr[:, :w], func=Relu,
                                     bias=fb2d[:])
                hr2s[c] = hr2

            def stage_out(c):
                w = 64 * cb(c)
                pr = prd.tile([128, 512], F32, tag="rd", name="prout")
                nc.tensor.matmul(out=pr[0:32, :w], lhsT=owdd[:],
                                 rhs=hr2s.pop(c)[:, :w], start=True, stop=True)
                o = rp.tile([32, 512], F32, tag="o", name="o")
                nc.scalar.activation(out=o[:, :w], in_=pr[0:32, :w],
                                     func=Relu, bias=obd[:])
                nc.sync.dma_start(out=oat_d[:, 512 * c:512 * c + w],
                                  in_=o[:, :w])

            # software-pipelined main loop (lags keep PE dependency-free)
            for i in range(cfg.NB + 12):
                if i % 8 == 0 and i // 8 - 1 < cfg.NCH and i >= 8:
                    stage_corr(i // 8 - 1)    # ordered before chunk's scatters
                if 0 <= i < cfg.NB:
                    stage_A(i)
                if 0 <= i - 2 < cfg.NB:
                    stage_B(i - 2)
                if 0 <= i - 4 < cfg.NB:
                    stage_C(i - 4)
                if 0 <= i - 8 < cfg.NB:
                    stage_E(i - 8)
                if 0 <= i - 6 < cfg.NB:
                    stage_D(i - 6)
                n_done = i - 8                # highest block whose scatter done
                for c in range(cfg.NCH):
                    lastb = min(8 * c + 7, cfg.NB - 1)
                    if n_done == lastb:
                        stage_nst(c)
                    elif n_done == lastb + 1:
                        stage_fc1(c)
                    elif n_done == lastb + 2:
                        stage_fc2(c)
                    elif n_done == lastb + 3:
                        stage_out(c)

    nc.compile()
    return nc


# ------------------------------------------------------------------ runner

_CACHE = {}


def _get_nc(cfg):
    key = (cfg.NB, cfg.APC)
    if key not in _CACHE:
        _CACHE[key] = build(cfg)
    return _CACHE[key]


def run(cfg, inputs, trace=False, tmpdir=None):
    ws = {k: np.asarray(v) for k, v in inputs.items()}
    x_bf = ws["atom_states"].astype(BF)
    shared = make_weight_inputs(ws)

    in_maps = []
    omaps = []
    for c in range(N_CORES):
        feat, smat, pads, omap = prep_core(
            cfg, c, x_bf, ws["edge_src"], ws["edge_dst"])
        m = dict(shared)
        m["feat"] = feat
        m["smat"] = smat
        m["pads"] = pads
        in_maps.append(m)
        omaps.append(omap)

    nc = _get_nc(cfg)
    kw = {}
    if trace:
        kw = dict(trace=True, tmpdir=tmpdir)
    r = run_bass_kernel_spmd(nc, in_maps, list(range(N_CORES)), **kw)

    out = np.zeros((cfg.MPC * N_CORES, OUT), np.float32)
    for c in range(N_CORES):
        oat = r.results[c]["oat"]                  # [32, NCH*512]
        o_row, o_col = omaps[c]
        per_atom = oat[o_row[:, None] + np.arange(OUT)[None, :], o_col[:, None]]
        out[c * cfg.MPC:(c + 1) * cfg.MPC] = \
            per_atom.reshape(cfg.MPC, ATOMS_PER_MOL, OUT).sum(1)
    return out, r


def kernel(**inputs) -> np.ndarray:
    out, _ = run(FULL, inputs)
    return out
